# revision 31
# baseline (speedup 1.0000x reference)
"""Differential multi-headed attention on 8 Trainium2 NeuronCores.

Sharding: core c = (batch b = c // 2, head-group g = c % 2).  Each core
computes 4 of the 8 differential heads for one batch element: Q/K/V
projections restricted to its 512 output dims, the differential
attention, and a partial (transposed) output projection.  The host sums
the two partial outputs per batch, un-permutes rows, and adds the output
bias (with the V-bias term folded in exactly: rows of p1 - lam*p2 sum
to 1 - lam, so bv contributes (1-lam)*bv@Wo^T, added host-side).

The reference's row-major [B,2dh,N,dk/2] reshape splits sub-heads over
the SEQUENCE axis: sub-head pair = seq halves, and within each half the
packed position index n2 = 2n + u couples seq n with dim-half u.  On
device we keep a blocked order n2' = u*512 + n (host permutes V rows to
match and un-permutes output rows), which needs a partition-swapped copy
of qt (qsw) so every (u_k, u_q) scores block reads lhsT and rhs from the
same base partitions.

Performance design: TRN2's tensor engine only reaches its full 2.4 GHz
clock after ~3us of *continuous* execution; any stall drops it back to
1.2 GHz.  So the kernel is one statically interleaved PE stream with no
intended gaps:

  - prologue: K/Q projections for head 0 (DMA-paced),
  - "h0" window: head-0 score chunks paced apart, with nearly all
    remaining projection work (K1-3, Q1-3, V m0-5) as PE filler while
    the scalar engine exps the head-0 scores,
  - windows w0-w2: [scores head j+1, AV-half head j] chunk pairs plus
    leftover V / softmax-denominator matmuls as filler,
  - w3: AV head 3 + output-projection partials for heads 0-2,
  - tail: head-3 output-projection contributions + drains.

Softmax denominators: DVE fold tree over the 8 exp chunks (bf16), a
ones-matmul partition reduction on the PE ([1,512] x2, ~0.4us), a fast
reciprocal straight out of PSUM, and a DRAM-bounce partition broadcast
(all with a full window of slack).  The differential combine is fused
with -lambda via scalar_tensor_tensor.

PSUM budget (8 banks): psS 2x[128,1024] (scores ring, reused by the
output projection) = 4, psO 2x[128,512] (AV half accumulators) = 2,
psF 2x[128,512] (projection half-groups, V chunks, Z tiles) = 2.
Matmul operands are bf16 (fp32 PSUM accumulation); fp32 elsewhere.
"""

import math
from contextlib import ExitStack

import ml_dtypes
import numpy as np

import concourse.bass as bass
import concourse.mybir as mybir
from concourse import bacc
import concourse.tile as tile
from concourse.bass_utils import run_bass_kernel_spmd

F32 = mybir.dt.float32
BF16 = mybir.dt.bfloat16
AF = mybir.ActivationFunctionType
ALU = mybir.AluOpType

N = 1024          # sequence length
D = 1024          # model dim
HG = 512          # head-group dims per core (4 heads x 128)
NHEAD = 4         # local heads per core
SCALE = 0.125     # 1/sqrt(dk/2) = 1/sqrt(64)
LAMBDA_INIT = 0.8

_BUILT = None     # cached Bass module -- building + compiling is expensive
LAST_RESULT = None  # BassKernelResults from the most recent run (for test.py)

# AV half-accumulation order: finish uq=0 (A) then uq=1 (B) for the low
# key chunks first so the A/B stop points land early enough to free the
# psO ring slots, while chunks 5-7 wait for V chunks finishing in w0.
AV_ORDER = [(0, 0), (0, 1), (0, 2), (0, 3), (0, 4),
            (1, 0), (1, 1), (1, 2), (1, 3), (1, 4),
            (0, 5), (0, 6), (0, 7), (1, 5), (1, 6), (1, 7)]


def _build():
    nc = bacc.Bacc()

    # ---- DRAM I/O (per core) ----
    xqT = nc.dram_tensor("xqT", [D, N], BF16, kind="ExternalInput")
    xkT = nc.dram_tensor("xkT", [D, N], BF16, kind="ExternalInput")
    xvT = nc.dram_tensor("xvT", [D, N], BF16, kind="ExternalInput")
    # per-head-packed projection weights: row j*128+p, col d*128 + c
    wqh = nc.dram_tensor("wqh", [512, 1024], BF16, kind="ExternalInput")
    wkh = nc.dram_tensor("wkh", [512, 1024], BF16, kind="ExternalInput")
    # V weights: [p, d*512 + c] (d = input-dim chunk, c = head-group dim)
    wvh = nc.dram_tensor("wvh", [128, 4096], BF16, kind="ExternalInput")
    # O weights: [p, j*1024 + c] (rows of Wo[:, sl].T per head)
    woh = nc.dram_tensor("woh", [128, 4096], BF16, kind="ExternalInput")
    bqp = nc.dram_tensor("bqp", [128, 4], F32, kind="ExternalInput")
    bkp = nc.dram_tensor("bkp", [128, 4], F32, kind="ExternalInput")
    lamneg = nc.dram_tensor("lamneg", [128, 1], F32, kind="ExternalInput")
    outT = nc.dram_tensor("outT", [D, N], F32, kind="ExternalOutput")

    with tile.TileContext(nc) as tc, ExitStack() as ctx:
        const = ctx.enter_context(tc.tile_pool(name="const", bufs=1))
        bq_sb = const.tile([128, 4], F32, name="bq_sb")
        bk_sb = const.tile([128, 4], F32, name="bk_sb")
        lam_sb = const.tile([128, 1], F32, name="lam_sb")
        ones_bf = const.tile([128, 1], BF16, name="ones_bf")
        warm = const.tile([1, 1], F32, name="warm")
        warm2 = const.tile([1, 1], F32, name="warm2")

        xst = ctx.enter_context(tc.tile_pool(name="xst", bufs=6))
        wqk = ctx.enter_context(tc.tile_pool(name="wqk", bufs=8))
        wvp = ctx.enter_context(tc.tile_pool(name="wvp", bufs=1))
        wop = ctx.enter_context(tc.tile_pool(name="wop", bufs=1))
        qkp = ctx.enter_context(tc.tile_pool(name="qkp", bufs=12))
        vvp = ctx.enter_context(tc.tile_pool(name="vvp", bufs=8))
        etp = ctx.enter_context(tc.tile_pool(name="etp", bufs=16))
        foldb = ctx.enter_context(tc.tile_pool(name="foldb", bufs=7))
        zrp = ctx.enter_context(tc.tile_pool(name="zrp", bufs=2))
        zp = ctx.enter_context(tc.tile_pool(name="zp", bufs=3))
        combp = ctx.enter_context(tc.tile_pool(name="combp", bufs=3))
        ohp = ctx.enter_context(tc.tile_pool(name="ohp", bufs=4))
        ostg = ctx.enter_context(tc.tile_pool(name="ostg", bufs=2))
        drb = ctx.enter_context(tc.tile_pool(name="drb", bufs=3, space="DRAM"))
        psS = ctx.enter_context(tc.tile_pool(name="psS", bufs=2, space="PSUM"))
        psO = ctx.enter_context(tc.tile_pool(name="psO", bufs=2, space="PSUM"))
        psF = ctx.enter_context(tc.tile_pool(name="psF", bufs=2, space="PSUM"))

        # x staging: two [128, 4096] tiles per tensor, one per SEQ half;
        # input-dim chunk d lives at cols d*512 (so a projection half-group
        # depends on a single staged tile)
        xk = [xst.tile([128, 4096], BF16, name=f"xk{h}", tag="x") for h in range(2)]
        xq = [xst.tile([128, 4096], BF16, name=f"xq{h}", tag="x") for h in range(2)]
        xv = [xst.tile([128, 4096], BF16, name=f"xv{h}", tag="x") for h in range(2)]
        wk_sb = [wqk.tile([128, N], BF16, name=f"wk{j}", tag="w") for j in range(4)]
        wq_sb = [wqk.tile([128, N], BF16, name=f"wq{j}", tag="w") for j in range(4)]
        wv_sb = wvp.tile([128, 4096], BF16, name="wv")
        wo_sb = wop.tile([128, 4096], BF16, name="wo")
        qt = [qkp.tile([128, N], BF16, name=f"qt{j}", tag="qk") for j in range(4)]
        kt = [qkp.tile([128, N], BF16, name=f"kt{j}", tag="qk") for j in range(4)]
        # partition-swapped copies of qt (see module docstring)
        qsw = [qkp.tile([128, N], BF16, name=f"qsw{j}", tag="qk") for j in range(4)]
        vv = [vvp.tile([128, HG], BF16, name=f"vv{m}", tag="v") for m in range(8)]
        oh = [ohp.tile([128, N], BF16, name=f"oh{j}", tag="oh") for j in range(4)]

        def xs(xt, half, d):
            return xt[half][:, d * 512:(d + 1) * 512]

        # ---- input DMAs, strict priority order on one queue (GPSIMD
        # sequencer; the sync queue is kept for SBUF-SBUF + output).
        # Large prologue-critical transfers first; the tiny bias tiles are
        # only needed at the first projection drain (~18us). ----
        # warm the sync-engine DMA path (its first transfer pays several
        # us of queue bring-up; the qsw copies must not eat that)
        nc.sync.dma_start(out=warm2[:], in_=lamneg[0:1, 0:1])
        nc.gpsimd.dma_start(out=wk_sb[0][:], in_=wkh[0:128, :])
        nc.gpsimd.dma_start(
            out=xk[0][:],
            in_=xkT[:, 0:512].rearrange("(d p) n -> p d n", p=128))
        nc.gpsimd.dma_start(out=wq_sb[0][:], in_=wqh[0:128, :])
        nc.gpsimd.dma_start(
            out=xq[0][:],
            in_=xqT[:, 0:512].rearrange("(d p) n -> p d n", p=128))
        nc.gpsimd.dma_start(
            out=xk[1][:],
            in_=xkT[:, 512:1024].rearrange("(d p) n -> p d n", p=128))
        nc.gpsimd.dma_start(
            out=xq[1][:],
            in_=xqT[:, 512:1024].rearrange("(d p) n -> p d n", p=128))
        nc.gpsimd.dma_start(out=bq_sb[:], in_=bqp[:, :])
        nc.gpsimd.dma_start(out=bk_sb[:], in_=bkp[:, :])
        nc.gpsimd.dma_start(out=lam_sb[:], in_=lamneg[:, :])
        for j in range(1, 4):
            nc.gpsimd.dma_start(out=wk_sb[j][:], in_=wkh[j * 128:(j + 1) * 128, :])
            nc.gpsimd.dma_start(out=wq_sb[j][:], in_=wqh[j * 128:(j + 1) * 128, :])
        for h in range(2):
            nc.gpsimd.dma_start(
                out=xv[h][:],
                in_=xvT[:, h * 512:(h + 1) * 512].rearrange("(d p) n -> p d n", p=128))
        nc.gpsimd.dma_start(out=wv_sb[:], in_=wvh[:, :])
        nc.gpsimd.dma_start(out=wo_sb[:], in_=woh[:, :])

        nc.vector.memset(ones_bf[:], 1.0)
        nc.vector.memset(warm[:], 0.0)
        # pre-load the scalar engine's Exp table during the DMA prologue
        nc.scalar.activation(warm[:], warm[:], AF.Exp)

        # ---------- filler generators (one PE matmul per yield) ----------
        def emit_proj_qk_half(j, half, w_sb, x_tiles, dst, bias_sb,
                              swap=None):
            """K/Q projection half-group: out dst[:, half*512:...]."""
            ps = psF.tile([128, 512], F32, name="pp", tag="pf")
            for d in range(8):
                nc.tensor.matmul(
                    ps[:],
                    w_sb[:, d * 128:(d + 1) * 128],
                    xs(x_tiles, half, d),
                    start=(d == 0), stop=(d == 7),
                )
                if d < 7:
                    yield
            nc.scalar.activation(
                dst[:, half * 512:(half + 1) * 512], ps[:], AF.Identity,
                bias=bias_sb[:, j:j + 1],
            )
            if swap is not None:
                # partition-swapped copy (engines cannot cross partitions;
                # stream_shuffle only permutes within 32-partition quadrants)
                cs = slice(half * 512, (half + 1) * 512)
                nc.sync.dma_start(out=swap[0:64, cs], in_=dst[64:128, cs])
                nc.sync.dma_start(out=swap[64:128, cs], in_=dst[0:64, cs])
            yield

        def emit_proj_q_head(j):
            yield from emit_proj_qk_half(j, 0, wq_sb[j], xq, qt[j], bq_sb,
                                         swap=qsw[j])
            yield from emit_proj_qk_half(j, 1, wq_sb[j], xq, qt[j], bq_sb,
                                         swap=qsw[j])

        def emit_proj_k_head(j):
            yield from emit_proj_qk_half(j, 0, wk_sb[j], xk, kt[j], bk_sb)
            yield from emit_proj_qk_half(j, 1, wk_sb[j], xk, kt[j], bk_sb)

        def emit_proj_v_chunk(m):
            """V projection for one (permuted) seq chunk m -> vv[m]."""
            ps = psF.tile([128, 512], F32, name="pv", tag="pf")
            for d in range(8):
                nc.tensor.matmul(
                    ps[:],
                    xs(xv, m // 4, d)[:, (m % 4) * 128:(m % 4) * 128 + 128],
                    wv_sb[:, d * 512:(d + 1) * 512],
                    start=(d == 0), stop=(d == 7),
                )
                if d < 7:
                    yield
            nc.vector.tensor_copy(out=vv[m][:], in_=ps[:])
            yield

        # ---------- per-(head, seq-half) attention state ----------
        etiles = {}   # (j, s) -> list of 8 et tiles
        fold_a = {}   # (j, s) -> partial fold tiles
        ff_t = {}     # (j, s) -> folded bf16 colsum-input tile (or b-pair)
        bcz = {}      # (j, s) -> broadcast reciprocal-colsum tile [128, N]
        SKIP_FF = {(3, 1)}   # tail-latency-critical sub-head

        def emit_score_chunk(j, s, kc):
            """Scores + exp + fold for chunk (head j, seq-half s, key block kc).

            Packed layout: score row m2 = w*512 + (kc%4)*128 + p covers key
            seq s*512 + (kc%4)*128 + p at dim-half w = kc//4; score col
            n2 = uq*512 + n covers query seq s*512 + n at dim-half uq.
            """
            w, mc = kc // 4, kc % 4
            lo = s * 512
            sp = psS.tile([128, N], F32, name="sp", tag="sp")
            et = etp.tile([128, N], BF16, name="et", tag="et")
            for uq in range(2):
                qsrc = qt[j] if uq == w else qsw[j]
                nc.tensor.matmul(
                    sp[:, uq * 512:(uq + 1) * 512],
                    kt[j][w * 64:(w + 1) * 64, lo + mc * 128:lo + (mc + 1) * 128],
                    qsrc[w * 64:(w + 1) * 64, lo:lo + 512],
                    start=True, stop=True,
                )
            nc.scalar.activation(et[:], sp[:], AF.Exp, scale=SCALE)
            key = (j, s)
            etiles.setdefault(key, []).append(et)
            # fold tree on DVE, all bf16
            if kc % 2 == 1:
                a = foldb.tile([128, N], BF16, name="fa", tag="f")
                nc.vector.tensor_tensor(
                    out=a[:], in0=etiles[key][kc - 1][:], in1=etiles[key][kc][:],
                    op=ALU.add)
                fold_a.setdefault(key, []).append(a)
            if kc == 3:
                b = foldb.tile([128, N], BF16, name="fb", tag="f")
                nc.vector.tensor_tensor(
                    out=b[:], in0=fold_a[key][0][:], in1=fold_a[key][1][:],
                    op=ALU.add)
                fold_a[key].append(b)
            if kc == 7:
                b = foldb.tile([128, N], BF16, name="fb", tag="f")
                nc.vector.tensor_tensor(
                    out=b[:], in0=fold_a[key][2][:], in1=fold_a[key][3][:],
                    op=ALU.add)
                if key in SKIP_FF:
                    # latency-critical (last sub-head): skip the final fold
                    # add, the Z ones-matmul accumulates over both b tiles
                    ff_t[key] = (fold_a[key][4], b)
                else:
                    ff = foldb.tile([128, N], BF16, name="ff", tag="f")
                    nc.vector.tensor_tensor(
                        out=ff[:], in0=fold_a[key][4][:], in1=b[:], op=ALU.add)
                    ff_t[key] = ff

        def emit_zchain(j, s):
            """Colsum of exp (partition reduce), reciprocal, broadcast.

            PE cost 2x512-col ones-matmuls; the broadcast bounces through
            DRAM on the sync queue.  Called with >= half a window of slack
            before bcz[(j, s)] is consumed.
            """
            ff = ff_t[(j, s)]
            srcs = ff if isinstance(ff, tuple) else (ff,)
            zrow = zrp.tile([1, N], F32, name="zr", tag="zr")
            for uq in range(2):
                psz = psF.tile([1, 512], F32, name="psz", tag="pf")
                for si, src in enumerate(srcs):
                    nc.tensor.matmul(
                        psz[:], ones_bf[:], src[:, uq * 512:(uq + 1) * 512],
                        start=(si == 0), stop=(si == len(srcs) - 1),
                    )
                nc.vector.reciprocal_approx_fast(
                    out=zrow[:, uq * 512:(uq + 1) * 512], in_=psz[:])
            zd = drb.tile([1, N], F32, name="zd", tag="zd")
            nc.sync.dma_start(out=zd[:], in_=zrow[:])
            z = zp.tile([128, N], F32, name="z", tag="z")
            nc.sync.dma_start(out=z[:], in_=zd[0, :].partition_broadcast(128))
            bcz[(j, s)] = z

        ops_t = {}    # (j, s, uq) -> PSUM half accumulator [128, 512]
        t_t = {}      # (j, term) -> t1 / t2 tile [128, N]

        def emit_av_half(j, s, uq, kc):
            if (j, s, uq) not in ops_t:
                ops_t[(j, s, uq)] = psO.tile([128, 512], F32, name="ops", tag="po")
            nc.tensor.matmul(
                ops_t[(j, s, uq)][:],
                vv[kc][:, j * 128:(j + 1) * 128],
                etiles[(j, s)][kc][:, uq * 512:(uq + 1) * 512],
                start=(kc == 0), stop=(kc == 7),
            )

        def emit_thalf(j, term, uq):
            """Normalize one AV half; term 0 = +s1/Z1, term 1 = -lam*s2/Z2."""
            if (j, term) not in t_t:
                t_t[(j, term)] = combp.tile([128, N], F32, name="t", tag="cb")
            t = t_t[(j, term)]
            sl_ = slice(uq * 512, (uq + 1) * 512)
            if term == 0:
                nc.vector.tensor_tensor(
                    out=t[:, sl_], in0=ops_t[(j, 0, uq)][:], in1=bcz[(j, 0)][:, sl_],
                    op=ALU.mult)
            else:
                nc.vector.scalar_tensor_tensor(
                    out=t[:, sl_], in0=ops_t[(j, 1, uq)][:], scalar=lam_sb[:],
                    in1=bcz[(j, 1)][:, sl_], op0=ALU.mult, op1=ALU.mult)

        def emit_combine(j):
            nc.vector.tensor_tensor(
                out=oh[j][:], in0=t_t[(j, 0)][:], in1=t_t[(j, 1)][:], op=ALU.add)

        po_t = {}

        def emit_oproj(oc, jlist, stop_j):
            if oc not in po_t:
                po_t[oc] = psS.tile([128, N], F32, name=f"po{oc}", tag="sp")
            po = po_t[oc]
            for j in jlist:
                for uh in range(2):
                    nc.tensor.matmul(
                        po[:, uh * 512:(uh + 1) * 512],
                        wo_sb[:, j * 1024 + oc * 128:j * 1024 + (oc + 1) * 128],
                        oh[j][:, uh * 512:(uh + 1) * 512],
                        start=(j == 0), stop=(j == stop_j),
                    )

        def emit_oproj_drain(oc):
            stg = ostg.tile([128, N], F32, name="stg", tag="og")
            nc.scalar.copy(stg[:], po_t[oc][:])
            nc.sync.dma_start(out=outT[oc * 128:(oc + 1) * 128, :], in_=stg[:])

        # ---------- P1: prologue, K0 + Q0 (DMA-paced) ----------
        for _ in emit_proj_k_head(0):
            pass
        for _ in emit_proj_q_head(0):
            pass

        # ---------- P2: h0 window -- head-0 scores + projection filler ----
        fillers = []
        for j in range(1, 4):
            fillers.append(emit_proj_k_head(j))
            fillers.append(emit_proj_q_head(j))
        for m in range(6):
            fillers.append(emit_proj_v_chunk(m))

        def pull_fillers(nmm):
            k = 0
            while k < nmm and fillers:
                try:
                    next(fillers[0])
                    k += 1
                except StopIteration:
                    fillers.pop(0)

        for i in range(16):
            s, kc = i // 8, i % 8
            emit_score_chunk(0, s, kc)
            pull_fillers(9)
            if i == 11:
                emit_zchain(0, 0)
        pull_fillers(10_000)  # flush remainder (144 total)

        # ---------- P3-P5: windows w0-w2 ----------
        # Per period: one score chunk of head j+1 (2 mm) + two AV half-
        # matmuls of head j.  Group stops land at periods 6 (s1A), 7.5
        # (s1B), 14 (s2A), 15.5 (s2B); emit_thalf right after each stop
        # frees the psO ring slot for the next group.
        av_stream = ([(0, uq, kc) for uq, kc in AV_ORDER]
                     + [(1, uq, kc) for uq, kc in AV_ORDER])
        for j in range(3):
            wf = []
            if j == 0:
                wf = [emit_proj_v_chunk(6), emit_proj_v_chunk(7)]
            for i in range(16):
                s, kc = i // 8, i % 8
                emit_score_chunk(j + 1, s, kc)
                for (avs, uq, akc) in av_stream[2 * i:2 * i + 2]:
                    emit_av_half(j, avs, uq, akc)
                # V filler (w0 only): 4 matmuls per early period
                k = 0
                while k < 4 and wf:
                    try:
                        next(wf[0])
                        k += 1
                    except StopIteration:
                        wf.pop(0)
                if i == 2 and j > 0:
                    emit_zchain(j, 1)      # Z2 of THIS head (exps from w_{j-1})
                if i == 5 and j == 0:
                    emit_zchain(0, 1)      # Z2 of head 0 (after V filler)
                if i == 11:
                    emit_zchain(j + 1, 0)  # Z1 of next head (s1 exps done)
                if i == 7:
                    emit_thalf(j, 0, 0)
                if i == 8:
                    emit_thalf(j, 0, 1)
                if i == 15:
                    emit_thalf(j, 1, 0)
            emit_thalf(j, 1, 1)
            emit_combine(j)

        # ---------- P6: w3 -- AV head 3 + output-projection partials.
        # The oc0/oc1/oc2 head-0..2 partials keep the PE fed while the
        # latency chain (last exp -> fold -> Z -> bounce -> thalf ->
        # combine) produces oh[3]. ----
        for i, (uq, kc) in enumerate(AV_ORDER):
            emit_av_half(3, 0, uq, kc)
            if i == 6:
                emit_zchain(3, 1)
        emit_thalf(3, 0, 0)
        emit_thalf(3, 0, 1)
        for (uq, kc) in AV_ORDER:
            emit_av_half(3, 1, uq, kc)
        emit_thalf(3, 1, 0)
        emit_thalf(3, 1, 1)
        emit_combine(3)
        emit_oproj(0, [0, 1, 2], stop_j=3)
        emit_oproj(1, [0, 1, 2], stop_j=3)
        # oc2 accumulates in two psF half-tiles (psF is free after the
        # last Z), giving 6 more filler matmuls before oh[3] is needed
        po2 = [psF.tile([128, 512], F32, name=f"po2{uh}", tag="pf")
               for uh in range(2)]
        for j in range(3):
            for uh in range(2):
                nc.tensor.matmul(
                    po2[uh][:],
                    wo_sb[:, j * 1024 + 2 * 128:j * 1024 + 3 * 128],
                    oh[j][:, uh * 512:(uh + 1) * 512],
                    start=(j == 0), stop=False,
                )

        # ---------- P7: head-3 contributions + remaining out chunks ----------
        emit_oproj(0, [3], stop_j=3)
        emit_oproj_drain(0)
        emit_oproj(1, [3], stop_j=3)
        emit_oproj_drain(1)
        for uh in range(2):
            nc.tensor.matmul(
                po2[uh][:],
                wo_sb[:, 3 * 1024 + 2 * 128:3 * 1024 + 3 * 128],
                oh[3][:, uh * 512:(uh + 1) * 512],
                start=False, stop=True,
            )
        stg2 = ostg.tile([128, N], F32, name="stg2", tag="og")
        for uh in range(2):
            nc.scalar.copy(stg2[:, uh * 512:(uh + 1) * 512], po2[uh][:])
        nc.sync.dma_start(out=outT[2 * 128:3 * 128, :], in_=stg2[:])
        for oc in range(3, 8):
            emit_oproj(oc, [0, 1, 2, 3], stop_j=3)
            emit_oproj_drain(oc)

    if not nc.is_finalized():
        nc.finalize()
    return nc


def _get_built():
    global _BUILT
    if _BUILT is None:
        _BUILT = _build()
    return _BUILT


def kernel(**inputs):
    inp = {k: np.asarray(v) for k, v in inputs.items()}
    q_, k_, v_ = inp["query"], inp["key"], inp["value"]
    Wq, Wk, Wv, Wo = inp["Wq"], inp["Wk"], inp["Wv"], inp["Wo"]
    bq_, bk_, bv_, bo_ = inp["bq"], inp["bk"], inp["bv"], inp["bo"]
    B = q_.shape[0]
    bf = ml_dtypes.bfloat16

    lam = (np.exp(np.sum(inp["lambda_q1"].astype(np.float64) * inp["lambda_k1"].astype(np.float64)))
           - np.exp(np.sum(inp["lambda_q2"].astype(np.float64) * inp["lambda_k2"].astype(np.float64)))
           + LAMBDA_INIT)

    def pack_qk(W, sl):
        # [512, 1024]: [j*128+p, d*128 + c] = W[sl].T[d*128+p, j*128+c]
        WT = np.ascontiguousarray(W[sl, :].T)  # [1024 in, 512 out]
        out = np.empty((512, 1024), dtype=bf)
        for j in range(4):
            blk = WT[:, j * 128:(j + 1) * 128].reshape(8, 128, 128)
            out[j * 128:(j + 1) * 128] = (
                blk.transpose(1, 0, 2).reshape(128, 1024).astype(bf))
        return out

    permv = np.arange(N).reshape(512, 2).T.reshape(-1)
    in_maps = []
    for c in range(8):
        b, g = c // 2, c % 2
        sl = slice(g * HG, (g + 1) * HG)
        WvT = np.ascontiguousarray(Wv[sl, :].T)  # [1024, 512]
        wvh = WvT.reshape(8, 128, 512).transpose(1, 0, 2).reshape(128, 4096)
        WoT = np.ascontiguousarray(Wo[:, sl].T)  # [512, 1024]
        woh = WoT.reshape(4, 128, 1024).transpose(1, 0, 2).reshape(128, 4096)
        in_maps.append({
            "xqT": np.ascontiguousarray(q_[b].T).astype(bf),
            "xkT": np.ascontiguousarray(k_[b].T).astype(bf),
            # value rows permuted: device row u*512+m <- seq 2m+u, matching
            # the packed key-block row order of the scores
            "xvT": np.ascontiguousarray(v_[b][permv].T).astype(bf),
            "wqh": pack_qk(Wq, sl),
            "wkh": pack_qk(Wk, sl),
            "wvh": np.ascontiguousarray(wvh).astype(bf),
            "woh": np.ascontiguousarray(woh).astype(bf),
            "bqp": np.ascontiguousarray(bq_[sl].reshape(4, 128).T).astype(np.float32),
            "bkp": np.ascontiguousarray(bk_[sl].reshape(4, 128).T).astype(np.float32),
            "lamneg": np.full((128, 1), -lam, dtype=np.float32),
        })

    nc = _get_built()
    res = run_bass_kernel_spmd(nc, in_maps, core_ids=list(range(8)))
    global LAST_RESULT
    LAST_RESULT = res

    # exact host-side fold of the V bias:  rows of (p1 - lam*p2) sum to 1-lam
    bo_eff = bo_ + (1.0 - lam) * (bv_.astype(np.float64) @ Wo.astype(np.float64).T)

    out = np.zeros((B, N, D), np.float32)
    for b in range(B):
        tot = res.results[2 * b]["outT"] + res.results[2 * b + 1]["outT"]
        # un-permute rows: device row u*512+n -> reference row 2n+u
        y = tot.T.reshape(2, 512, D).transpose(1, 0, 2).reshape(N, D)
        out[b] = y + bo_eff.astype(np.float32)
    return out
